# revision 1
# baseline (speedup 1.0000x reference)
"""Trainium2 Bass kernel for a single-timestep custom LSTM cell.

Math (per reference):
    gates = x @ Wx^T + h_prev @ Wh^T + bias          [B, 4H]
    f,i,o = sigmoid(gates_f/i/o);  c_tilde = tanh(gates_c)
    mask  = (||x_row||_2 > 1e-3)                      per batch row
    c_next = (f + i) * c_prev + mask * (i * c_tilde)
    h_next = o * tanh(c_next)
    returns (h_next, c_next, c_tilde)

Strategy: 8-way data parallel over the batch dim (512 rows/core), weights
replicated. Per core the GEMM contracts over the input dim, which is the
inner (free) dim of both x and W in DRAM — so both operands are transposed
on-chip with PE-transpose (exact, fp32) and the PSUM->SBUF copies round to
float32r (TF32) so the main matmuls run at full PE rate (1 cyc/row at
N=512). Bias is folded in as an extra K=1 matmul against a ones vector.
"""

import sys

sys.path.insert(0, "/opt/trn_rl_repo")

import numpy as np

import concourse.bass as bass
import concourse.mybir as mybir
import concourse.tile as tile
from concourse import bacc
from concourse.masks import make_identity

B, I, H = 4096, 1024, 1024
NCORES = 8
BS = B // NCORES  # 512 batch rows per core
G4 = 4 * H  # 4096
F32 = mybir.dt.float32
F32R = mybir.dt.float32r
ACTF = mybir.ActivationFunctionType
ALU = mybir.AluOpType


def _build_nc():
    nc = bacc.Bacc(trn_type="TRN2", enable_partition_id=False)
    x_d = nc.dram_tensor("x", [BS, I], F32, kind="ExternalInput")
    h_d = nc.dram_tensor("h", [BS, H], F32, kind="ExternalInput")
    c_d = nc.dram_tensor("c", [BS, H], F32, kind="ExternalInput")
    wx_d = nc.dram_tensor("wx", [G4, I], F32, kind="ExternalInput")
    wh_d = nc.dram_tensor("wh", [G4, H], F32, kind="ExternalInput")
    bias_d = nc.dram_tensor("bias", [1, G4], F32, kind="ExternalInput")
    hn_d = nc.dram_tensor("h_next", [BS, H], F32, kind="ExternalOutput")
    cn_d = nc.dram_tensor("c_next", [BS, H], F32, kind="ExternalOutput")
    ct_d = nc.dram_tensor("c_tilde", [BS, H], F32, kind="ExternalOutput")

    NB = BS // 128  # 4 batch tiles per core
    KI = I // 128  # 8 k-tiles on the x side
    KH = H // 128  # 8 k-tiles on the h side
    KT = KI + KH  # 16 contraction tiles

    with tile.TileContext(nc) as tc:
        with (
            tc.tile_pool(name="const", bufs=1) as const,
            tc.tile_pool(name="stage", bufs=8) as stage,
            tc.tile_pool(name="resident", bufs=1) as resident,
            tc.tile_pool(name="wt", bufs=1) as wtp,
            tc.tile_pool(name="gates", bufs=1) as gatesp,
            tc.tile_pool(name="outs", bufs=2) as outs,
            tc.tile_pool(name="ps_mm", bufs=3, space="PSUM") as ps_mm,
            tc.tile_pool(name="ps_tr", bufs=3, space="PSUM") as ps_tr,
        ):
            ident = const.tile([128, 128], F32)
            make_identity(nc, ident)

            ones_f = const.tile([1, 128], F32)
            nc.vector.memset(ones_f, 1.0)
            ones_r = const.tile([1, 128], F32R)
            nc.vector.tensor_copy(out=ones_r, in_=ones_f)

            bias_f = const.tile([1, G4], F32)
            nc.sync.dma_start(out=bias_f, in_=bias_d[:, :])
            bias_r = const.tile([1, G4], F32R)
            nc.vector.tensor_copy(out=bias_r, in_=bias_f)

            # c_prev resident, natural layout [128, bt, H]
            c_sb = resident.tile([128, NB, H], F32)
            nc.sync.dma_start(
                out=c_sb, in_=c_d.rearrange("(bt p) h -> p bt h", p=128)
            )

            mask_sb = const.tile([128, NB], F32)
            sq_scratch = const.tile([128, I], F32)

            # Transpose x and h into f32r lhsT tiles: [128(i), bt, k, 128(b)]
            xT = resident.tile([128, NB, KI, 128], F32R)
            hT = resident.tile([128, NB, KH, 128], F32R)
            for src_d, dstT, nk in ((x_d, xT, KI), (h_d, hT, KH)):
                for bt in range(NB):
                    s_nat = stage.tile([128, I], F32, tag="stage")
                    nc.sync.dma_start(
                        out=s_nat, in_=src_d[bt * 128 : (bt + 1) * 128, :]
                    )
                    if src_d is x_d:
                        # row L2 norm^2 via Square activation w/ accumulate
                        sumsq = const.tile([128, 1], F32, tag="sumsq")
                        nc.scalar.activation(
                            out=sq_scratch,
                            in_=s_nat,
                            func=ACTF.Square,
                            accum_out=sumsq,
                        )
                        nc.vector.tensor_scalar(
                            out=mask_sb[:, bt : bt + 1],
                            in0=sumsq,
                            scalar1=1e-6,
                            scalar2=None,
                            op0=ALU.is_gt,
                        )
                    for kg in range(nk // 4):
                        pt = ps_tr.tile([128, 512], F32, tag="pt")
                        for j in range(4):
                            ko = kg * 4 + j
                            nc.tensor.transpose(
                                pt[:, j * 128 : (j + 1) * 128],
                                s_nat[:, ko * 128 : (ko + 1) * 128],
                                ident,
                            )
                        nc.vector.tensor_copy(
                            out=dstT[:, bt, kg * 4 : (kg + 1) * 4, :], in_=pt
                        )

            # main loop: two column-halves (s), four gates (f,i,o,c)
            for s in range(2):
                gtiles = []
                for g in range(4):
                    n0 = g * H + s * 512
                    # stage W rows [n0:n0+512] for both wx and wh
                    wt_t = wtp.tile([128, KT, 512], F32R, tag="wt")
                    for side, (w_d, kbase) in enumerate(
                        ((wx_d, 0), (wh_d, KI))
                    ):
                        wstg = []
                        for p in range(4):
                            t = stage.tile([128, I], F32, tag="stage")
                            nc.sync.dma_start(
                                out=t,
                                in_=w_d[n0 + p * 128 : n0 + (p + 1) * 128, :],
                            )
                            wstg.append(t)
                        for ko in range(KI):
                            pt = ps_tr.tile([128, 512], F32, tag="pt")
                            for p in range(4):
                                nc.tensor.transpose(
                                    pt[:, p * 128 : (p + 1) * 128],
                                    wstg[p][:, ko * 128 : (ko + 1) * 128],
                                    ident,
                                )
                            # alternate copy engine to balance DVE/ACT load
                            if ko % 2 == 0:
                                nc.vector.tensor_copy(
                                    out=wt_t[:, kbase + ko, :], in_=pt
                                )
                            else:
                                nc.scalar.copy(
                                    out=wt_t[:, kbase + ko, :], in_=pt
                                )

                    gt = gatesp.tile([128, NB, 512], F32, tag=f"g{g}")
                    gtiles.append(gt)
                    for bt in range(NB):
                        pg = ps_mm.tile([128, 512], F32, tag="pg")
                        for k in range(KT):
                            lhs = (
                                xT[:, bt, k, :]
                                if k < KI
                                else hT[:, bt, k - KI, :]
                            )
                            nc.tensor.matmul(
                                pg,
                                lhs,
                                wt_t[:, k, :],
                                start=(k == 0),
                                stop=False,
                            )
                        nc.tensor.matmul(
                            pg,
                            ones_r,
                            bias_r[:, n0 : n0 + 512],
                            start=False,
                            stop=True,
                        )
                        nc.scalar.activation(
                            out=gt[:, bt, :],
                            in_=pg,
                            func=ACTF.Tanh if g == 3 else ACTF.Sigmoid,
                        )

                # elementwise combine for this column-half
                gf, gi, go, gc = gtiles
                for bt in range(NB):
                    f_ = gf[:, bt, :]
                    i_ = gi[:, bt, :]
                    o_ = go[:, bt, :]
                    ct_ = gc[:, bt, :]
                    cp_ = c_sb[:, bt, s * 512 : (s + 1) * 512]
                    t_fi = outs.tile([128, 512], F32, tag="t_fi")
                    nc.vector.tensor_add(t_fi, f_, i_)
                    t2 = outs.tile([128, 512], F32, tag="t2")
                    nc.vector.tensor_mul(t2, t_fi, cp_)
                    t3 = outs.tile([128, 512], F32, tag="t3")
                    nc.vector.scalar_tensor_tensor(
                        out=t3,
                        in0=i_,
                        scalar=mask_sb[:, bt : bt + 1],
                        in1=ct_,
                        op0=ALU.mult,
                        op1=ALU.mult,
                    )
                    cn = outs.tile([128, 512], F32, tag="cn")
                    nc.vector.tensor_add(cn, t2, t3)
                    tn = outs.tile([128, 512], F32, tag="tn")
                    nc.scalar.activation(out=tn, in_=cn, func=ACTF.Tanh)
                    hn = outs.tile([128, 512], F32, tag="hn")
                    nc.vector.tensor_mul(hn, o_, tn)
                    row = slice(bt * 128, (bt + 1) * 128)
                    col = slice(s * 512, (s + 1) * 512)
                    nc.sync.dma_start(out=cn_d[row, col], in_=cn)
                    nc.sync.dma_start(out=hn_d[row, col], in_=hn)
                    nc.sync.dma_start(out=ct_d[row, col], in_=ct_)

    nc.finalize()
    return nc


_RUNNER = None


def _get_runner():
    global _RUNNER
    if _RUNNER is not None:
        return _RUNNER

    import jax
    from jax.sharding import Mesh, PartitionSpec
    from jax.experimental.shard_map import shard_map
    from concourse.bass2jax import (
        _bass_exec_p,
        install_neuronx_cc_hook,
    )

    install_neuronx_cc_hook()
    nc = _build_nc()

    in_names = ["x", "h", "c", "wx", "wh", "bias"]
    sharded_in = {"x", "h", "c"}  # split on batch; weights replicated
    out_names = ["h_next", "c_next", "c_tilde"]
    out_shapes = {name: (BS, H) for name in out_names}
    out_avals = [
        jax.core.ShapedArray(out_shapes[n], np.float32) for n in out_names
    ]

    def _body(*args):
        outs = _bass_exec_p.bind(
            *args,
            out_avals=tuple(out_avals),
            in_names=tuple(in_names + out_names),
            out_names=tuple(out_names),
            lowering_input_output_aliases=(),
            sim_require_finite=True,
            sim_require_nnan=True,
            nc=nc,
        )
        return tuple(outs)

    devices = jax.devices()[:NCORES]
    mesh = Mesh(np.asarray(devices), ("core",))
    in_specs = tuple(
        PartitionSpec("core") if n in sharded_in else PartitionSpec()
        for n in in_names
    ) + (PartitionSpec("core"),) * len(out_names)
    out_specs = (PartitionSpec("core"),) * len(out_names)
    n_in = len(in_names)
    donate = tuple(range(n_in, n_in + len(out_names)))
    jitted = jax.jit(
        shard_map(
            _body, mesh=mesh, in_specs=in_specs, out_specs=out_specs,
            check_rep=False,
        ),
        donate_argnums=donate,
        keep_unused=True,
    )

    def run(x, h, c, wx, wh, bias):
        zeros = [
            np.zeros((NCORES * BS, H), np.float32) for _ in out_names
        ]
        outs = jitted(x, h, c, wx, wh, bias, *zeros)
        return tuple(np.asarray(o) for o in outs)

    _RUNNER = run
    return run


def kernel(
    x, h_prev, c_prev, c_prev_tilde_dummy,
    Wf, bWf, Vf, bVf, bf,
    Wi, bWi, Vi, bVi, bi,
    Wo, bWo, Vo, bVo, bo,
    Wc, bWc, Vc, bVc, bc,
):
    run = _get_runner()
    f32 = np.float32
    x = np.ascontiguousarray(np.asarray(x, f32))
    h = np.ascontiguousarray(np.asarray(h_prev, f32))
    c = np.ascontiguousarray(np.asarray(c_prev, f32))
    wx = np.ascontiguousarray(
        np.concatenate([Wf, Wi, Wo, Wc], axis=0).astype(f32)
    )
    wh = np.ascontiguousarray(
        np.concatenate([Vf, Vi, Vo, Vc], axis=0).astype(f32)
    )
    bias = (
        np.concatenate([bWf, bWi, bWo, bWc])
        + np.concatenate([bVf, bVi, bVo, bVc])
        + np.concatenate([bf, bi, bo, bc])
    ).astype(f32).reshape(1, G4)
    bias = np.ascontiguousarray(bias)

    h_next, c_next, c_tilde = run(x, h, c, wx, wh, bias)
    return h_next, c_next, c_tilde


# revision 11
# speedup vs baseline: 22509.9192x; 22509.9192x over previous
"""Trainium2 Bass kernel for a single-timestep custom LSTM cell.

Math (per reference):
    gates = x @ Wx^T + h_prev @ Wh^T + bias          [B, 4H]
    f,i,o = sigmoid(gates_f/i/o);  c_tilde = tanh(gates_c)
    mask  = (||x_row||_2 > 1e-3)                      per batch row
    c_next = (f + i) * c_prev + mask * (i * c_tilde)
    h_next = o * tanh(c_next)
    returns (h_next, c_next, c_tilde)

Strategy: 8-way data parallel over the batch dim (512 rows/core), weights
replicated. Per core the GEMM contracts over the input dim, which is the
inner (free) dim of both x and W in DRAM — so both operands are transposed
on-chip with PE-transpose (exact, fp32) and the PSUM->SBUF copies round to
float32r (TF32) so the main matmuls run at full PE rate (1 cyc/row at
N=512). Bias is folded in as an extra K=1 matmul against a ones vector.
"""

import sys

sys.path.insert(0, "/opt/trn_rl_repo")

import numpy as np

import concourse.bass as bass
import concourse.mybir as mybir
import concourse.tile as tile
from concourse import bacc
from concourse.masks import make_identity

B, I, H = 4096, 1024, 1024
NCORES = 8
BS = B // NCORES  # 512 batch rows per core
G4 = 4 * H  # 4096
F32 = mybir.dt.float32
F32R = mybir.dt.float32r
ACTF = mybir.ActivationFunctionType
ALU = mybir.AluOpType


def _build_nc(reps=1, skip_wtr=False, skip_mm=False):
    """Build the per-core Bass program. reps>1 wraps the whole body in an
    on-device loop (used only for device-time measurement). skip_wtr /
    skip_mm produce wrong results and exist only for timing attribution."""
    nc = bacc.Bacc(trn_type="TRN2", enable_partition_id=False)
    x_d = nc.dram_tensor("x", [BS, I], F32, kind="ExternalInput")
    h_d = nc.dram_tensor("h", [BS, H], F32, kind="ExternalInput")
    c_d = nc.dram_tensor("c", [BS, H], F32, kind="ExternalInput")
    wx_d = nc.dram_tensor("wx", [G4, I], F32, kind="ExternalInput")
    wh_d = nc.dram_tensor("wh", [G4, H], F32, kind="ExternalInput")
    bias_d = nc.dram_tensor("bias", [1, G4], F32, kind="ExternalInput")
    hn_d = nc.dram_tensor("h_next", [BS, H], F32, kind="ExternalOutput")
    cn_d = nc.dram_tensor("c_next", [BS, H], F32, kind="ExternalOutput")
    ct_d = nc.dram_tensor("c_tilde", [BS, H], F32, kind="ExternalOutput")

    NB = BS // 128  # 4 batch tiles per core
    KI = I // 128  # 8 k-tiles on the x side
    KH = H // 128  # 8 k-tiles on the h side
    KT = KI + KH  # 16 contraction tiles

    from contextlib import ExitStack, nullcontext

    with tile.TileContext(nc) as tc, ExitStack() as ctx:
        loop = tc.For_i(0, reps) if reps > 1 else nullcontext()
        with loop:
            const = ctx.enter_context(tc.tile_pool(name="const", bufs=1))
            stage = ctx.enter_context(tc.tile_pool(name="stage", bufs=8))
            resident = ctx.enter_context(tc.tile_pool(name="resident", bufs=1))
            wtp = ctx.enter_context(tc.tile_pool(name="wt", bufs=1))
            gatesp = ctx.enter_context(tc.tile_pool(name="gates", bufs=1))
            outs = ctx.enter_context(tc.tile_pool(name="outs", bufs=2))
            ps_mm = ctx.enter_context(
                tc.tile_pool(name="ps_mm", bufs=3, space="PSUM")
            )
            ps_tr = ctx.enter_context(
                tc.tile_pool(name="ps_tr", bufs=3, space="PSUM")
            )
            ident = const.tile([128, 128], F32)
            make_identity(nc, ident)

            ones_f = const.tile([1, 128], F32)
            nc.vector.memset(ones_f, 1.0)
            ones_r = const.tile([1, 128], F32R)
            nc.vector.tensor_copy(out=ones_r, in_=ones_f)

            bias_f = const.tile([1, G4], F32)
            nc.sync.dma_start(out=bias_f, in_=bias_d[:, :])
            bias_r = const.tile([1, G4], F32R)
            nc.vector.tensor_copy(out=bias_r, in_=bias_f)

            # c_prev resident, natural layout [128, bt, H]
            c_sb = resident.tile([128, NB, H], F32)
            nc.sync.dma_start(
                out=c_sb, in_=c_d.rearrange("(bt p) h -> p bt h", p=128)
            )

            mask_sb = const.tile([128, NB], F32)
            sq_scratch = const.tile([128, I], F32)

            # Transpose x and h into f32r lhsT tiles: [128(i), bt, k, 128(b)]
            xT = resident.tile([128, NB, KI, 128], F32R)
            hT = resident.tile([128, NB, KH, 128], F32R)
            for src_d, dstT, nk in ((x_d, xT, KI), (h_d, hT, KH)):
                for bt in range(NB):
                    s_nat = stage.tile([128, I], F32, tag="stage")
                    nc.sync.dma_start(
                        out=s_nat, in_=src_d[bt * 128 : (bt + 1) * 128, :]
                    )
                    if src_d is x_d:
                        # row L2 norm^2 via Square activation w/ accumulate
                        sumsq = const.tile([128, 1], F32, tag="sumsq")
                        nc.scalar.activation(
                            out=sq_scratch,
                            in_=s_nat,
                            func=ACTF.Square,
                            accum_out=sumsq,
                        )
                        nc.vector.tensor_scalar(
                            out=mask_sb[:, bt : bt + 1],
                            in0=sumsq,
                            scalar1=1e-6,
                            scalar2=None,
                            op0=ALU.is_gt,
                        )
                    for kg in range(nk // 4):
                        pt = ps_tr.tile([128, 512], F32, tag="pt")
                        for j in range(4):
                            ko = kg * 4 + j
                            nc.tensor.transpose(
                                pt[:, j * 128 : (j + 1) * 128],
                                s_nat[:, ko * 128 : (ko + 1) * 128],
                                ident,
                            )
                        nc.vector.tensor_copy(
                            out=dstT[:, bt, kg * 4 : (kg + 1) * 4, :], in_=pt
                        )

            # main loop: two column-halves (s), four gates (f,i,o,c)
            for s in range(2):
                gtiles = []
                for g in range(4):
                    n0 = g * H + s * 512
                    # stage W rows [n0:n0+512] for both wx and wh
                    wt_t = wtp.tile([128, KT, 512], F32R, tag="wt")
                    for side, (w_d, kbase) in enumerate(
                        ((wx_d, 0), (wh_d, KI))
                    ):
                        wstg = []
                        for p in range(4):
                            t = stage.tile([128, I], F32, tag="stage")
                            nc.sync.dma_start(
                                out=t,
                                in_=w_d[n0 + p * 128 : n0 + (p + 1) * 128, :],
                            )
                            wstg.append(t)
                        if skip_wtr:
                            # timing-only: copies without PE transposes
                            for ko in range(KI):
                                src = wstg[ko % 4][
                                    :, (ko % 2) * 512 : (ko % 2) * 512 + 512
                                ]
                                if ko % 2 == 0:
                                    nc.vector.tensor_copy(
                                        out=wt_t[:, kbase + ko, :], in_=src
                                    )
                                else:
                                    nc.scalar.copy(
                                        out=wt_t[:, kbase + ko, :], in_=src
                                    )
                            continue
                        for ko in range(KI):
                            pt = ps_tr.tile([128, 512], F32, tag="pt")
                            for p in range(4):
                                nc.tensor.transpose(
                                    pt[:, p * 128 : (p + 1) * 128],
                                    wstg[p][:, ko * 128 : (ko + 1) * 128],
                                    ident,
                                )
                            # alternate copy engine to balance DVE/ACT load
                            if ko % 2 == 0:
                                nc.vector.tensor_copy(
                                    out=wt_t[:, kbase + ko, :], in_=pt
                                )
                            else:
                                nc.scalar.copy(
                                    out=wt_t[:, kbase + ko, :], in_=pt
                                )

                    gt = gatesp.tile([128, NB, 512], F32, tag=f"g{g}")
                    gtiles.append(gt)
                    for bt in range(NB):
                        pg = ps_mm.tile([128, 512], F32, tag="pg")
                        if skip_mm:
                            nc.tensor.matmul(
                                pg,
                                ones_r,
                                bias_r[:, n0 : n0 + 512],
                                start=True,
                                stop=True,
                            )
                        else:
                            for k in range(KT):
                                lhs = (
                                    xT[:, bt, k, :]
                                    if k < KI
                                    else hT[:, bt, k - KI, :]
                                )
                                nc.tensor.matmul(
                                    pg,
                                    lhs,
                                    wt_t[:, k, :],
                                    start=(k == 0),
                                    stop=False,
                                )
                            nc.tensor.matmul(
                                pg,
                                ones_r,
                                bias_r[:, n0 : n0 + 512],
                                start=False,
                                stop=True,
                            )
                        nc.scalar.activation(
                            out=gt[:, bt, :],
                            in_=pg,
                            func=ACTF.Tanh if g == 3 else ACTF.Sigmoid,
                        )

                # elementwise combine for this column-half
                gf, gi, go, gc = gtiles
                for bt in range(NB):
                    f_ = gf[:, bt, :]
                    i_ = gi[:, bt, :]
                    o_ = go[:, bt, :]
                    ct_ = gc[:, bt, :]
                    cp_ = c_sb[:, bt, s * 512 : (s + 1) * 512]
                    t_fi = outs.tile([128, 512], F32, tag="t_fi")
                    nc.vector.tensor_add(t_fi, f_, i_)
                    t2 = outs.tile([128, 512], F32, tag="t2")
                    nc.vector.tensor_mul(t2, t_fi, cp_)
                    t3 = outs.tile([128, 512], F32, tag="t3")
                    nc.vector.scalar_tensor_tensor(
                        out=t3,
                        in0=i_,
                        scalar=mask_sb[:, bt : bt + 1],
                        in1=ct_,
                        op0=ALU.mult,
                        op1=ALU.mult,
                    )
                    cn = outs.tile([128, 512], F32, tag="cn")
                    nc.vector.tensor_add(cn, t2, t3)
                    tn = outs.tile([128, 512], F32, tag="tn")
                    nc.scalar.activation(out=tn, in_=cn, func=ACTF.Tanh)
                    hn = outs.tile([128, 512], F32, tag="hn")
                    nc.vector.tensor_mul(hn, o_, tn)
                    row = slice(bt * 128, (bt + 1) * 128)
                    col = slice(s * 512, (s + 1) * 512)
                    nc.sync.dma_start(out=cn_d[row, col], in_=cn)
                    nc.sync.dma_start(out=hn_d[row, col], in_=hn)
                    nc.sync.dma_start(out=ct_d[row, col], in_=ct_)

    nc.finalize()
    return nc


_JITTED = {}

IN_NAMES = ["x", "h", "c", "wx", "wh", "bias"]
SHARDED_IN = {"x", "h", "c"}  # split on batch; weights replicated
OUT_NAMES = ["h_next", "c_next", "c_tilde"]


def _get_jitted(reps=1, **build_kwargs):
    """Jitted runner for the bass program built with `reps` on-device
    repetitions of the body. reps=1 is the normal path; reps>1 is used for
    device-time measurement (slope over reps)."""
    key = (reps, tuple(sorted(build_kwargs.items())))
    if key in _JITTED:
        return _JITTED[key]

    import jax
    from jax.sharding import Mesh, PartitionSpec
    from jax.experimental.shard_map import shard_map
    from concourse.bass2jax import (
        _bass_exec_p,
        install_neuronx_cc_hook,
    )

    install_neuronx_cc_hook()
    nc = _build_nc(reps=reps, **build_kwargs)

    out_avals = [
        jax.core.ShapedArray((BS, H), np.float32) for _ in OUT_NAMES
    ]

    def _body(*args):
        outs = _bass_exec_p.bind(
            *args,
            out_avals=tuple(out_avals),
            in_names=tuple(IN_NAMES + OUT_NAMES),
            out_names=tuple(OUT_NAMES),
            lowering_input_output_aliases=(),
            sim_require_finite=True,
            sim_require_nnan=True,
            nc=nc,
        )
        return tuple(outs)

    devices = jax.devices()[:NCORES]
    mesh = Mesh(np.asarray(devices), ("core",))
    in_specs = tuple(
        PartitionSpec("core") if n in SHARDED_IN else PartitionSpec()
        for n in IN_NAMES
    ) + (PartitionSpec("core"),) * len(OUT_NAMES)
    out_specs = (PartitionSpec("core"),) * len(OUT_NAMES)
    n_in = len(IN_NAMES)
    donate = tuple(range(n_in, n_in + len(OUT_NAMES)))
    jitted = jax.jit(
        shard_map(
            _body, mesh=mesh, in_specs=in_specs, out_specs=out_specs,
            check_rep=False,
        ),
        donate_argnums=donate,
        keep_unused=True,
    )
    _JITTED[key] = jitted
    return jitted


def _get_runner():
    jitted = _get_jitted(1)

    def run(x, h, c, wx, wh, bias):
        zeros = [
            np.zeros((NCORES * BS, H), np.float32) for _ in OUT_NAMES
        ]
        outs = jitted(x, h, c, wx, wh, bias, *zeros)
        return tuple(np.asarray(o) for o in outs)

    return run


def kernel(
    x, h_prev, c_prev, c_prev_tilde_dummy,
    Wf, bWf, Vf, bVf, bf,
    Wi, bWi, Vi, bVi, bi,
    Wo, bWo, Vo, bVo, bo,
    Wc, bWc, Vc, bVc, bc,
):
    run = _get_runner()
    f32 = np.float32
    x = np.ascontiguousarray(np.asarray(x, f32))
    h = np.ascontiguousarray(np.asarray(h_prev, f32))
    c = np.ascontiguousarray(np.asarray(c_prev, f32))
    wx = np.ascontiguousarray(
        np.concatenate([Wf, Wi, Wo, Wc], axis=0).astype(f32)
    )
    wh = np.ascontiguousarray(
        np.concatenate([Vf, Vi, Vo, Vc], axis=0).astype(f32)
    )
    bias = (
        np.concatenate([bWf, bWi, bWo, bWc])
        + np.concatenate([bVf, bVi, bVo, bVc])
        + np.concatenate([bf, bi, bo, bc])
    ).astype(f32).reshape(1, G4)
    bias = np.ascontiguousarray(bias)

    h_next, c_next, c_tilde = run(x, h, c, wx, wh, bias)
    return h_next, c_next, c_tilde
